# revision 4
# baseline (speedup 1.0000x reference)
# DiabaticReadout forward on Trainium2 (Bass/Tile), 8-core data-parallel.
#
# Per sample i: H = [[d0, lam], [lam, d1]] (2x2 symmetric).  Eigenvalues in
# closed form:
#   mean    = 0.5*(d0+d1)
#   halfgap = sqrt(0.25*((d0-d1)^2 + 4*lam^2))
#   e0, e1  = mean -/+ halfgap          (ascending, matches eigh)
#
# Purely elementwise -> shard the N axis across the 8 NeuronCores, each core
# streams [128, F] tiles: 3 input DMAs, 5 DVE ops, 3 ACT ops, 1 output DMA.
# The 0.5 factors are folded for free: ACT's activation computes
# func(scale*in + bias), so Square(lam, scale=2) = 4*lam^2 and
# Sqrt(s, scale=0.25) = 0.5*sqrt(s); the final mean-/+halfgap pair uses the
# fused DVE scalar_tensor_tensor: (sum * 0.5) -/+ halfgap, written straight
# into an interleaved [128, F, 2] tile so the store is one contiguous DMA.

import numpy as np

import concourse.tile as tile
from concourse import bacc, mybir
from concourse.bass_utils import run_bass_kernel_spmd

N_CORES = 8
P = 128  # SBUF partitions

_cache = {}


def _build(rows, f_tile):
    """Build the per-core Bass module: inputs [P*rows] f32, output [P*rows, 2]."""
    C = P * rows
    f32 = mybir.dt.float32
    Alu = mybir.AluOpType
    Act = mybir.ActivationFunctionType

    nc = bacc.Bacc(
        "TRN2",
        target_bir_lowering=False,
        debug=False,
        num_devices=N_CORES,
    )
    d0 = nc.dram_tensor("d0", [C], f32, kind="ExternalInput").ap()
    d1 = nc.dram_tensor("d1", [C], f32, kind="ExternalInput").ap()
    lam = nc.dram_tensor("lam", [C], f32, kind="ExternalInput").ap()
    out = nc.dram_tensor("out", [C, 2], f32, kind="ExternalOutput").ap()

    d0v = d0.rearrange("(p f) -> p f", p=P)
    d1v = d1.rearrange("(p f) -> p f", p=P)
    lamv = lam.rearrange("(p f) -> p f", p=P)
    outv = out.rearrange("(p f) two -> p f two", p=P)

    with tile.TileContext(nc) as tc:
        with (
            tc.tile_pool(name="io", bufs=3) as io,
            tc.tile_pool(name="tmp", bufs=2) as tmp,
        ):
            f0 = 0
            while f0 < rows:
                F = min(f_tile, rows - f0)
                sl = slice(f0, f0 + F)

                t_d0 = io.tile([P, F], f32, tag="d0")
                nc.sync.dma_start(t_d0[:], d0v[:, sl])
                t_d1 = io.tile([P, F], f32, tag="d1")
                nc.sync.dma_start(t_d1[:], d1v[:, sl])
                t_lam = io.tile([P, F], f32, tag="lam")
                nc.sync.dma_start(t_lam[:], lamv[:, sl])

                t_sum = tmp.tile([P, F], f32, tag="sum")
                nc.vector.tensor_add(t_sum[:], t_d0[:], t_d1[:])
                t_dif = tmp.tile([P, F], f32, tag="dif")
                nc.vector.tensor_sub(t_dif[:], t_d0[:], t_d1[:])

                t_l2 = tmp.tile([P, F], f32, tag="l2")
                nc.scalar.activation(t_l2[:], t_lam[:], Act.Square, scale=2.0)
                t_d2 = tmp.tile([P, F], f32, tag="d2")
                nc.scalar.activation(t_d2[:], t_dif[:], Act.Square)

                t_s = tmp.tile([P, F], f32, tag="s")
                nc.vector.tensor_add(t_s[:], t_d2[:], t_l2[:])
                t_r = tmp.tile([P, F], f32, tag="r")
                nc.scalar.activation(t_r[:], t_s[:], Act.Sqrt, scale=0.25)

                t_out = io.tile([P, F, 2], f32, tag="out")
                nc.vector.scalar_tensor_tensor(
                    t_out[:, :, 0], t_sum[:], 0.5, t_r[:], Alu.mult, Alu.subtract
                )
                nc.vector.scalar_tensor_tensor(
                    t_out[:, :, 1], t_sum[:], 0.5, t_r[:], Alu.mult, Alu.add
                )
                nc.sync.dma_start(outv[:, sl, :], t_out[:])

                f0 += F
    nc.compile()
    return nc


def _get_nc(rows, f_tile):
    key = (rows, f_tile)
    if key not in _cache:
        _cache[key] = _build(rows, f_tile)
    return _cache[key]


def kernel(d0, d1, lam, _trace=False, _f_tile=1536):
    d0 = np.ascontiguousarray(np.asarray(d0), dtype=np.float32).ravel()
    d1 = np.ascontiguousarray(np.asarray(d1), dtype=np.float32).ravel()
    lam = np.ascontiguousarray(np.asarray(lam), dtype=np.float32).ravel()
    n = d0.shape[0]

    # Per-core sample count: multiple of 128, cores cover ceil(n / 8).
    rows = -(-n // (N_CORES * P))  # ceil
    C = P * rows
    total = N_CORES * C
    pad = total - n
    if pad:
        z = np.zeros(pad, np.float32)
        d0 = np.concatenate([d0, z])
        d1 = np.concatenate([d1, z])
        lam = np.concatenate([lam, z])

    in_maps = [
        {
            "d0": np.ascontiguousarray(d0[c * C : (c + 1) * C]),
            "d1": np.ascontiguousarray(d1[c * C : (c + 1) * C]),
            "lam": np.ascontiguousarray(lam[c * C : (c + 1) * C]),
        }
        for c in range(N_CORES)
    ]

    nc = _get_nc(rows, _f_tile)
    res = run_bass_kernel_spmd(
        nc, in_maps, core_ids=list(range(N_CORES)), trace=_trace
    )
    global last_results
    last_results = res
    full = np.concatenate([res.results[c]["out"] for c in range(N_CORES)], axis=0)
    return full[:n]


last_results = None
